# revision 3
# baseline (speedup 1.0000x reference)
"""Multi-head causal attention (B=2, S=4096, D=768, H=12) on 8 Trainium2 cores, v2.

Sharding: one (batch, 3-head group) per core (as v1).  Changes vs v1:
  - score matmuls run as PE row-tile pairs (heads 0/1 cross-paired via
    kh01/qh01 partition halves; head 2 self-paired via a duplicated
    qh2d/kh2d [128, sq] produced by col-tiled projection matmuls)
  - PV uses fp8e4m3 DoubleRow over sk-tile pairs (keeps the ones-column
    denominator trick, M=65); diagonal tiles stay single fp8 matmuls
  - Wo uses fp8 DoubleRow over the head-0/1 pair; head 2 plain fp8
  - scale folding: Wv x64 (lifts feats out of e4m3 denormals), Wo x16;
    host divides partials by 1024
  - block 0 (queries 0..511, where |out| is ~20x larger) runs a bf16
    path end-to-end; fp8 only covers rows >= 512 where its ~4% relative
    error is far below the tolerance
  - block-level software pipelining: projection and Wo matmul chunks are
    interleaved between attention pair-iterations (generator feeder), so
    the ACT engine (softmax exp, the bottleneck at ~1 elem/lane/cycle)
    never waits on a serial projection phase
"""

import numpy as np
import ml_dtypes

USE_DR = True  # fp8 DoubleRow for PV + Wo

import concourse.bass as bass
import concourse.mybir as mybir
from concourse.tile import TileContext
from bass_rust import ScopedClock

B, S, D, H = 2, 4096, 768, 12
HD = D // H  # 64
N_CORES = 8
CORES_PER_BATCH = 4
HPC = H // CORES_PER_BATCH  # heads per core = 3
HB = HPC * HD  # head-block width = 192
SQ = 512  # query-column block (matmul moving free dim)
F32 = mybir.dt.float32
BF16 = mybir.dt.bfloat16
FP8 = mybir.dt.float8e4
AF = mybir.ActivationFunctionType
DR = mybir.MatmulPerfMode.DoubleRow
WV_SCALE = 64.0
WO_SCALE = 16.0
OUT_SCALE = WV_SCALE * WO_SCALE


class PatchedTileContext(TileContext):
    """This walrus build encodes at most 2 sync-waits per CTRL instruction,
    but the stock kernel-tail drain carries one wait per active proc.
    Distribute the waits across single-wait NOPs ahead of the drain."""

    def _drain_and_barrier(self, tick_clock, wait_clock):
        probe = self.nc.sync.nop(nofuse=True, hint="drain_waits").ins
        wait_clock.add_sem_waits(probe, ScopedClock({None: tick_clock.global_clock}))
        waits = list(probe.sync_info.on_wait) if probe.sync_info else []
        updates = list(probe.sync_info.on_update) if probe.sync_info else []
        probe.sync_info = mybir.SyncInfo(on_wait=waits[:1], on_update=updates)
        for k in range(1, len(waits)):
            nxt = self.nc.sync.nop(nofuse=True, hint=f"drain_waits_{k}").ins
            nxt.sync_info = mybir.SyncInfo(on_wait=[waits[k]], on_update=[])
        self.nc.sync.drain()
        self.nc.all_engine_barrier()
        popped = self.nc._tile_sem_poison_stack.pop()
        assert popped is self._sem_poison
        self.nc.clear_and_free_semaphores(list(self.sems.allocated().values()))
        self.nc.all_engine_barrier()


def build_program(nc, s_total=S, pss_bufs=3, pt_bufs=4, feats_bufs=2, time_reps=1):
    """Emit the per-core attention program. s_total must divide by 512."""
    nb = s_total // SQ
    qT = nc.dram_tensor("qT", [D, s_total], BF16, kind="ExternalInput")
    kT = nc.dram_tensor("kT", [D, s_total], BF16, kind="ExternalInput")
    vT = nc.dram_tensor("vT", [D, s_total], BF16, kind="ExternalInput")
    wqT = nc.dram_tensor("wqT", [D, HB], BF16, kind="ExternalInput")
    wkT = nc.dram_tensor("wkT", [D, HB], BF16, kind="ExternalInput")
    wvT = nc.dram_tensor("wvT", [D, HB], BF16, kind="ExternalInput")
    wo8 = nc.dram_tensor("wo8", [64, 2, D], FP8, kind="ExternalInput")
    wo28 = nc.dram_tensor("wo28", [64, D], FP8, kind="ExternalInput")
    woTb = nc.dram_tensor("woTb", [HB, D], BF16, kind="ExternalInput")
    bias_qk = nc.dram_tensor("bias_qk", [128, 2, 2], F32, kind="ExternalInput")
    bv_row = nc.dram_tensor("bv_row", [1, HB], BF16, kind="ExternalInput")
    ones128 = nc.dram_tensor("ones128", [1, 128], BF16, kind="ExternalInput")
    out = nc.dram_tensor("out", [s_total, D], F32, kind="ExternalOutput")

    with PatchedTileContext(nc) as tc:
        import contextlib

        with contextlib.ExitStack() as ctx:
            cpool = ctx.enter_context(tc.tile_pool(name="consts", bufs=1))
            stream = ctx.enter_context(tc.tile_pool(name="stream", bufs=3))
            qh_pool = ctx.enter_context(tc.tile_pool(name="qh", bufs=2))
            kv_pool = ctx.enter_context(tc.tile_pool(name="kv", bufs=2))
            pt_pool = ctx.enter_context(tc.tile_pool(name="pt", bufs=pt_bufs))
            sm_pool = ctx.enter_context(tc.tile_pool(name="sm", bufs=2))
            feats_pool = ctx.enter_context(tc.tile_pool(name="feats", bufs=feats_bufs))
            osb_pool = ctx.enter_context(tc.tile_pool(name="osb", bufs=2))
            # PSUM budget (8 banks): sc2 3x[128,2,512] (6) + pf 2x[65,512] (2).
            ps_sc2 = ctx.enter_context(
                tc.tile_pool(name="ps_sc2", bufs=pss_bufs, space="PSUM")
            )
            ps_feat = ctx.enter_context(tc.tile_pool(name="ps_feat", bufs=2, space="PSUM"))

            # ---- constants / weights ----
            wq_sb = cpool.tile([128, 6, HB], BF16, tag="wq")
            wk_sb = cpool.tile([128, 6, HB], BF16, tag="wk")
            wv_sb = cpool.tile([128, 6, HB], BF16, tag="wv")
            for dst, src in ((wq_sb, wqT), (wk_sb, wkT), (wv_sb, wvT)):
                nc.sync.dma_start(
                    out=dst[:], in_=src[:].rearrange("(c p) m -> p c m", p=128)
                )
            wo_sb = cpool.tile([64, 2, D], FP8, tag="wo")
            nc.sync.dma_start(out=wo_sb[:], in_=wo8[:])
            wo2_sb = cpool.tile([64, D], FP8, tag="wo2")
            nc.sync.dma_start(out=wo2_sb[:], in_=wo28[:])
            # bf16 Wo (x16) for the block-0 bf16 path
            wob_sb = cpool.tile([64, HPC, D], BF16, tag="wob")
            nc.sync.dma_start(out=wob_sb[:], in_=woTb[:].rearrange("(h p) n -> p h n", p=64))
            bias_sb = cpool.tile([128, 2, 2], F32, tag="bias")
            nc.sync.dma_start(out=bias_sb[:], in_=bias_qk[:])
            bv_sb = cpool.tile([1, HB], BF16, tag="bv")
            nc.sync.dma_start(out=bv_sb[:], in_=bv_row[:])
            ones_row = cpool.tile([1, 128], BF16, tag="ones")
            nc.sync.dma_start(out=ones_row[:], in_=ones128[:])
            # 0/1 causal masks for the 4 diagonal sk-tiles of a 512 block:
            # mask_m[p, f] = 1 where f >= p + 128*m else 0
            masks = cpool.tile([128, 4, SQ], BF16, tag="masks")
            nc.gpsimd.memset(masks[:], 0.0)
            for m in range(4):
                nc.gpsimd.affine_select(
                    out=masks[:, m, :],
                    in_=masks[:, m, :],
                    compare_op=mybir.AluOpType.is_gt,
                    fill=1.0,
                    base=128 * m,
                    pattern=[[-1, SQ]],
                    channel_multiplier=1,
                )

            for _rep in range(time_reps):
                kh01s = []  # [128, SQ] bf16 per block (heads 0,1 on partition halves)
                kh2s = []  # [128, SQ] bf16 per block (head 2 duplicated)
                vhs = []  # [128, 2(sp), 3(h), 2(ko), 80] fp8 per block (V', ones col)
                qhs = {}  # j -> (qh01, qh2d), dropped after use
                vjb_cell = [None]  # bf16 V' for block 0

                def dma_block(j):
                    sq_lo = j * SQ
                    qt = stream.tile([128, 6, SQ], BF16, tag="qt")
                    kt = stream.tile([128, 6, SQ], BF16, tag="kt")
                    vt = stream.tile([128, 6, SQ], BF16, tag="vt")
                    for dst, src in ((qt, qT), (kt, kT), (vt, vT)):
                        nc.sync.dma_start(
                            out=dst[:],
                            in_=src[:].rearrange("(c p) s -> p c s", p=128)[
                                :, :, sq_lo : sq_lo + SQ
                            ],
                        )
                    return qt, kt, vt

                def proj_block(j, qt, kt, vt):
                    """Generator: Q/K/V' projections for block j, yielding
                    between small PE chunks so attention exps can overlap."""
                    qh01 = qh_pool.tile([128, SQ], BF16, tag="qh01")
                    qh2d = qh_pool.tile([128, SQ], BF16, tag="qh2d")
                    qhs[j] = (qh01, qh2d)
                    k01 = kv_pool.tile([128, SQ], BF16, tag=f"kh01_{j % 8}")
                    k2d = kv_pool.tile([128, SQ], BF16, tag=f"kh2d_{j % 8}")
                    kh01s.append(k01)
                    kh2s.append(k2d)
                    for xt, wsb, o01, o2d, bi in (
                        (qt, wq_sb, qh01, qh2d, 0),
                        (kt, wk_sb, k01, k2d, 1),
                    ):
                        ps = ps_sc2.tile([128, 2, SQ], F32, tag="sc2")
                        for c in range(6):
                            nc.tensor.matmul(
                                ps[:, 0, :],
                                lhsT=wsb[:, c, 0:128],
                                rhs=xt[:, c, :],
                                start=(c == 0),
                                stop=(c == 5),
                            )
                            # col-tiled pair: head-2 weights into both halves
                            nc.tensor.matmul(
                                ps[0:64, 1, :],
                                lhsT=wsb[:, c, 128:HB],
                                rhs=xt[:, c, :],
                                start=(c == 0),
                                stop=(c == 5),
                            )
                            nc.tensor.matmul(
                                ps[64:128, 1, :],
                                lhsT=wsb[:, c, 128:HB],
                                rhs=xt[:, c, :],
                                start=(c == 0),
                                stop=(c == 5),
                            )
                            if c % 2 == 1:
                                yield
                        nc.vector.tensor_scalar_add(o01[:], ps[:, 0, :], bias_sb[:, bi, 0:1])
                        nc.vector.tensor_scalar_add(o2d[:], ps[:, 1, :], bias_sb[:, bi, 1:2])
                        yield
                    vj = kv_pool.tile([128, 2, HPC, 2, 80], FP8, tag=f"vh_{j % 8}")
                    vjb = None
                    if j == 0:  # bf16 V' for the block-0 bf16 path
                        vjb = kv_pool.tile([128, 4, HPC, HD + 1], BF16, tag="vhb")
                        vjb_cell[0] = vjb
                    for sp in range(2):
                        psv2 = ps_sc2.tile([128, 2, SQ], F32, tag="sc2")
                        for ko in range(2):
                            st = 2 * sp + ko
                            psv = psv2[:, ko, 0:HB]
                            for c in range(6):
                                nc.tensor.matmul(
                                    psv,
                                    lhsT=vt[:, c, st * 128 : (st + 1) * 128],
                                    rhs=wv_sb[:, c, :],
                                    start=(c == 0),
                                    stop=False,
                                )
                            nc.tensor.matmul(
                                psv, lhsT=ones_row[:], rhs=bv_sb[:], start=False, stop=True
                            )
                            if j == 0:
                                nc.vector.tensor_copy(
                                    out=vjb[:, st, :, 0:HD],
                                    in_=psv.rearrange("p (h e) -> p h e", e=HD),
                                )
                                nc.vector.memset(vjb[:, st, :, HD : HD + 1], 1.0)
                            nc.vector.tensor_copy(
                                out=vj[:, sp, :, ko, 0:HD],
                                in_=psv.rearrange("p (h e) -> p h e", e=HD),
                            )
                            nc.vector.memset(vj[:, sp, :, ko, HD : HD + 1], 1.0)
                            yield
                    vhs.append(vj)

                def wo_block(j, feats, feats2):
                    """Generator: output projection for block j, one st per chunk."""
                    sq_lo = j * SQ
                    for st in range(4):
                        pso = ps_sc2.tile([128, 2, SQ], F32, tag="sc2")
                        for i1, n0, nsz in ((0, 0, 512), (1, 512, 256)):
                            if j != 0 and USE_DR:
                                nc.tensor.matmul(
                                    pso[:, i1, 0:nsz],
                                    lhsT=feats[:, :, st * 128 : (st + 1) * 128],
                                    rhs=wo_sb[:, :, n0 : n0 + nsz],
                                    start=True,
                                    stop=False,
                                    perf_mode=DR,
                                )
                            elif j != 0:
                                for h in (0, 1):
                                    nc.tensor.matmul(
                                        pso[:, i1, 0:nsz],
                                        lhsT=feats[:, h, st * 128 : (st + 1) * 128],
                                        rhs=wo_sb[:, h, n0 : n0 + nsz],
                                        start=(h == 0),
                                        stop=False,
                                    )
                            else:  # bf16 path: heads 0,1 separately
                                for h in (0, 1):
                                    nc.tensor.matmul(
                                        pso[:, i1, 0:nsz],
                                        lhsT=feats[:, h, st * 128 : (st + 1) * 128],
                                        rhs=wob_sb[:, h, n0 : n0 + nsz],
                                        start=(h == 0),
                                        stop=False,
                                    )
                            nc.tensor.matmul(
                                pso[:, i1, 0:nsz],
                                lhsT=feats2[:, st * 128 : (st + 1) * 128],
                                rhs=(wo2_sb if j != 0 else wob_sb[:, 2, :])[
                                    :, n0 : n0 + nsz
                                ],
                                start=False,
                                stop=True,
                            )
                        osb = osb_pool.tile([128, D], F32, tag="osb")
                        nc.vector.tensor_copy(
                            out=osb[:], in_=pso[:].rearrange("p a b -> p (a b)")[:, 0:D]
                        )
                        nc.sync.dma_start(
                            out=out[sq_lo + st * 128 : sq_lo + (st + 1) * 128, :], in_=osb[:]
                        )
                        yield

                def kslice(ksb, half, t):
                    jj, tt = t // 4, t % 4
                    return ksb[jj][64 * half : 64 * (half + 1), tt * 128 : (tt + 1) * 128]

                def attention_block(j, feeder):
                    """Attention for block j; pulls feeder chunks between u's."""
                    n_u = 2 * (j + 1)
                    fp8 = j != 0
                    ptdt = FP8 if fp8 else BF16
                    if fp8:
                        feats = feats_pool.tile([64, 2, SQ], FP8, tag="feats01")
                        feats2 = feats_pool.tile([64, SQ], FP8, tag="feats2")
                    else:
                        feats = feats_pool.tile([64, 2, SQ], BF16, tag="feats01b")
                        feats2 = feats_pool.tile([64, SQ], BF16, tag="feats2b")
                    qh01, qh2d = qhs.pop(j)
                    # 2 phases x n_u iterations; feeder chunks spread evenly
                    total_steps = 2 * n_u

                    def pull(step):
                        want = ((step + 1) * feeder_len) // total_steps
                        while pull.done < want:
                            try:
                                next(feeder)
                            except StopIteration:
                                pull.done = feeder_len
                                return
                            pull.done += 1

                    pull.done = 0
                    feeder_len = feeder_est

                    # --- phase A: heads 0,1 cross-paired ---
                    psf_a = ps_feat.tile([HD + 1, SQ], F32, tag="pf")
                    psf_b = ps_feat.tile([HD + 1, SQ], F32, tag="pf")
                    psf01 = [psf_a, psf_b]
                    for u in range(n_u):
                        t0 = 2 * u
                        if u < n_u - 2:  # off-diagonal pair
                            pta = pt_pool.tile([128, 2, 2, SQ], FP8, tag="pta")
                            for i1 in (0, 1):
                                t = t0 + i1
                                pss = ps_sc2.tile([128, 2, SQ], F32, tag="sc2")
                                for h in (0, 1):
                                    nc.tensor.matmul(
                                        pss[:, h, :],
                                        lhsT=kslice(kh01s, h, t),
                                        rhs=qh01[64 * h : 64 * (h + 1), :],
                                        start=True,
                                        stop=True,
                                    )
                                nc.scalar.activation(
                                    pta[:, i1, :, :], pss[:], AF.Exp, bias=0.0, scale=0.125
                                )
                            jj, sp = t0 // 4, (t0 % 4) // 2
                            for h in (0, 1):
                                if USE_DR:
                                    nc.tensor.matmul(
                                        psf01[h][:, :],
                                        lhsT=vhs[jj][:, sp, h, :, 0 : HD + 1],
                                        rhs=pta[:, :, h, :],
                                        start=(u == 0),
                                        stop=False,
                                        perf_mode=DR,
                                    )
                                else:
                                    for ko in (0, 1):
                                        nc.tensor.matmul(
                                            psf01[h][:, :],
                                            lhsT=vhs[jj][:, sp, h, ko, 0 : HD + 1],
                                            rhs=pta[:, ko, h, :],
                                            start=(u == 0 and ko == 0),
                                            stop=False,
                                        )
                        else:  # diagonal tiles, one at a time
                            for m in (t0 - 4 * j, t0 - 4 * j + 1):
                                t = 4 * j + m
                                lo = 128 * m
                                ptd = pt_pool.tile([128, 2, SQ], ptdt, tag="ptd")
                                pss = ps_sc2.tile([128, 2, SQ], F32, tag="sc2")
                                for h in (0, 1):
                                    nc.tensor.matmul(
                                        pss[:, h, lo:SQ],
                                        lhsT=kslice(kh01s, h, t),
                                        rhs=qh01[64 * h : 64 * (h + 1), lo:SQ],
                                        start=True,
                                        stop=True,
                                    )
                                nc.scalar.activation(
                                    ptd[:, :, lo:SQ], pss[:, :, lo:SQ],
                                    AF.Exp, bias=0.0, scale=0.125,
                                )
                                jj, sp, ko = t // 4, (t % 4) // 2, t % 2
                                for h in (0, 1):
                                    nc.vector.tensor_mul(
                                        ptd[:, h, lo:SQ], ptd[:, h, lo:SQ],
                                        masks[:, m, lo:SQ],
                                    )
                                    lv = (
                                        vhs[jj][:, sp, h, ko, 0 : HD + 1]
                                        if fp8
                                        else vjb_cell[0][:, m, h, 0 : HD + 1]
                                    )
                                    nc.tensor.matmul(
                                        psf01[h][:, lo:SQ],
                                        lhsT=lv,
                                        rhs=ptd[:, h, lo:SQ],
                                        start=(j == 0 and m == 0),
                                        stop=(m == 3),
                                    )
                        pull(u)

                    # normalize heads 0,1 now: frees their pf buffers so
                    # psf2 (third "pf" allocation) only waits on these muls
                    for h in (0, 1):
                        recip = sm_pool.tile([1, SQ], F32, tag="recip")
                        nc.vector.reciprocal(recip[:], psf01[h][HD : HD + 1, :])
                        rbc = sm_pool.tile([64, SQ], F32, tag="rbc")
                        nc.gpsimd.partition_broadcast(rbc[:], recip[:])
                        nc.vector.tensor_mul(feats[:, h, :], psf01[h][0:HD, :], rbc[:])

                    # --- phase B: head 2 self-paired via duplicated halves ---
                    psf2 = ps_feat.tile([HD + 1, SQ], F32, tag="pf")
                    for u in range(n_u):
                        t0 = 2 * u
                        if u < n_u - 2:  # off-diagonal pair
                            ptc = pt_pool.tile([128, 2, SQ], FP8, tag="ptc")
                            pss = ps_sc2.tile([128, 2, SQ], F32, tag="sc2")
                            for i1 in (0, 1):
                                nc.tensor.matmul(
                                    pss[:, i1, :],
                                    lhsT=kslice(kh2s, i1, t0 + i1),
                                    rhs=qh2d[64 * i1 : 64 * (i1 + 1), :],
                                    start=True,
                                    stop=True,
                                )
                            nc.scalar.activation(
                                ptc[:], pss[:], AF.Exp, bias=0.0, scale=0.125
                            )
                            jj, sp = t0 // 4, (t0 % 4) // 2
                            if USE_DR:
                                nc.tensor.matmul(
                                    psf2[:, :],
                                    lhsT=vhs[jj][:, sp, 2, :, 0 : HD + 1],
                                    rhs=ptc[:, :, :],
                                    start=(u == 0),
                                    stop=False,
                                    perf_mode=DR,
                                )
                            else:
                                for ko in (0, 1):
                                    nc.tensor.matmul(
                                        psf2[:, :],
                                        lhsT=vhs[jj][:, sp, 2, ko, 0 : HD + 1],
                                        rhs=ptc[:, ko, :],
                                        start=(u == 0 and ko == 0),
                                        stop=False,
                                    )
                        else:  # diagonal pair (m0, m1) with different widths
                            m0 = t0 - 4 * j
                            ptc = pt_pool.tile([128, 2, SQ], ptdt, tag="ptc")
                            pss = ps_sc2.tile([128, 2, SQ], F32, tag="sc2")
                            for i1 in (0, 1):
                                m = m0 + i1
                                t = 4 * j + m
                                lo = 128 * m
                                nc.tensor.matmul(
                                    pss[:, i1, lo:SQ],
                                    lhsT=kslice(kh2s, i1, t),
                                    rhs=qh2d[64 * i1 : 64 * (i1 + 1), lo:SQ],
                                    start=True,
                                    stop=True,
                                )
                                nc.scalar.activation(
                                    ptc[:, i1, lo:SQ], pss[:, i1, lo:SQ],
                                    AF.Exp, bias=0.0, scale=0.125,
                                )
                                nc.vector.tensor_mul(
                                    ptc[:, i1, lo:SQ], ptc[:, i1, lo:SQ],
                                    masks[:, m, lo:SQ],
                                )
                                jj, sp, ko = t // 4, (t % 4) // 2, t % 2
                                lv = (
                                    vhs[jj][:, sp, 2, ko, 0 : HD + 1]
                                    if fp8
                                    else vjb_cell[0][:, m, 2, 0 : HD + 1]
                                )
                                nc.tensor.matmul(
                                    psf2[:, lo:SQ],
                                    lhsT=lv,
                                    rhs=ptc[:, i1, lo:SQ],
                                    start=(j == 0 and m == 0),
                                    stop=(m == 3),
                                )
                        pull(n_u + u)

                    # drain any feeder remainder
                    for _ in feeder:
                        pass

                    # --- normalize head 2 -> fp8 feats ---
                    recip = sm_pool.tile([1, SQ], F32, tag="recip")
                    nc.vector.reciprocal(recip[:], psf2[HD : HD + 1, :])
                    rbc = sm_pool.tile([64, SQ], F32, tag="rbc")
                    nc.gpsimd.partition_broadcast(rbc[:], recip[:])
                    nc.vector.tensor_mul(feats2[:], psf2[0:HD, :], rbc[:])
                    return feats, feats2

                # ---- software-pipelined block loop ----
                import itertools

                feeder_est = 0  # recomputed per block below
                tiles = dma_block(0)
                for _ in proj_block(0, *tiles):
                    pass
                prev_feats = None
                for j in range(nb):
                    gens = []
                    if prev_feats is not None:
                        gens.append(wo_block(j - 1, *prev_feats))
                    if j + 1 < nb:
                        tiles = dma_block(j + 1)
                        gens.append(proj_block(j + 1, *tiles))
                    feeder_est = (4 if prev_feats is not None else 0) + (
                        12 if j + 1 < nb else 0
                    )
                    prev_feats = attention_block(j, itertools.chain(*gens))
                for _ in wo_block(nb - 1, *prev_feats):
                    pass

    return nc


def build_nc(s_total=S, **kw):
    from concourse import bacc

    nc = bacc.Bacc(num_devices=N_CORES)
    build_program(nc, s_total=s_total, **kw)
    nc.compile()
    return nc


# ---------------------------------------------------------------------------
# Host-side sharding / unsharding
# ---------------------------------------------------------------------------


def shard_inputs(q, k, v, Wq, bq, Wk, bk, Wv, bv, Wo, bo, s_total=S):
    """Build the 8 per-core input maps (numpy)."""
    in_maps = []
    qT = [np.ascontiguousarray(np.asarray(q)[b, :s_total].T) for b in range(B)]
    kTb = [np.ascontiguousarray(np.asarray(k)[b, :s_total].T) for b in range(B)]
    vTb = [np.ascontiguousarray(np.asarray(v)[b, :s_total].T) for b in range(B)]
    Wq, Wk, Wv, Wo = (np.asarray(x) for x in (Wq, Wk, Wv, Wo))
    bq, bk, bv = (np.asarray(x) for x in (bq, bk, bv))
    for c in range(N_CORES):
        b = c // CORES_PER_BATCH
        g = c % CORES_PER_BATCH
        lo, hi = HB * g, HB * (g + 1)
        bias_qk = np.zeros((128, 2, 2), np.float32)
        for i, bvec in enumerate((bq[lo:hi], bk[lo:hi])):
            bias_qk[:128, i, 0] = bvec[:128]
            bias_qk[:64, i, 1] = bvec[128:]
            bias_qk[64:128, i, 1] = bvec[128:]
        # Wo: heads 0,1 -> [64, 2, D]; head 2 -> [64, D]; both x16 fp8
        woc = Wo[:, lo:hi].T * WO_SCALE  # [192, D]
        wo8 = np.ascontiguousarray(
            woc[:128].reshape(2, 64, D).transpose(1, 0, 2)
        ).astype(ml_dtypes.float8_e4m3)
        wo28 = np.ascontiguousarray(woc[128:]).astype(ml_dtypes.float8_e4m3)
        in_maps.append(
            {
                "qT": qT[b].astype(ml_dtypes.bfloat16),
                "kT": kTb[b].astype(ml_dtypes.bfloat16),
                "vT": vTb[b].astype(ml_dtypes.bfloat16),
                "wqT": np.ascontiguousarray(Wq[lo:hi].T).astype(ml_dtypes.bfloat16),
                "wkT": np.ascontiguousarray(Wk[lo:hi].T).astype(ml_dtypes.bfloat16),
                "wvT": np.ascontiguousarray(Wv[lo:hi].T * WV_SCALE).astype(
                    ml_dtypes.bfloat16
                ),
                "wo8": wo8,
                "wo28": wo28,
                "woTb": np.ascontiguousarray(woc).astype(ml_dtypes.bfloat16),
                "bias_qk": bias_qk,
                "bv_row": np.ascontiguousarray(bv[lo:hi] * WV_SCALE)[None, :].astype(
                    ml_dtypes.bfloat16
                ),
                "ones128": np.ones((1, 128), ml_dtypes.bfloat16),
            }
        )
    return in_maps


def unshard_outputs(results, bo, s_total=S):
    """Sum the 4 row-parallel partials per batch, unscale, and add bo."""
    bo = np.asarray(bo, np.float32)
    out = np.empty((B, s_total, D), np.float32)
    for b in range(B):
        acc = results[b * CORES_PER_BATCH]["out"].astype(np.float32)
        for c in range(b * CORES_PER_BATCH + 1, (b + 1) * CORES_PER_BATCH):
            acc = acc + results[c]["out"]
        out[b] = acc / OUT_SCALE + bo
    return out


def kernel(q, k, v, mask, Wq, bq, Wk, bk, Wv, bv, Wo, bo):
    """Full-input entry point: returns [B, S, D] float32."""
    from concourse.bass_utils import run_bass_kernel_spmd

    nc = build_nc()
    in_maps = shard_inputs(q, k, v, Wq, bq, Wk, bk, Wv, bv, Wo, bo)
    res = run_bass_kernel_spmd(nc, in_maps, list(range(N_CORES)))
    return unshard_outputs(res.results, bo)
